# revision 6
# baseline (speedup 1.0000x reference)
"""Causal self-attention Trainium2 kernel (B=2, T=2048, C=1024, H=16).

Sharding: 8 cores = 2 batches x 4 head-groups (4 heads/core, Megatron-style
column-parallel QKV + row-parallel proj; the row-parallel all-reduce is the
host-side partial sum in `kernel`).

Structure: QKV(t5+1) projection blocks are interleaved into the
attention(t5) group loop so the PE fills its exp-wait stalls with QKV work
(all PSUM accumulators rotate through one unified 2-slot pool + ytps + proj
slots = exactly 8 banks). The scores->exp->AV chain is software-pipelined
with lookahead-2 (one strided ACT per k-block covering both packed heads).
Causal masking is folded into the scores PSUM accumulation as an
identity-stationary matmul adding -30 to the upper triangle. Softmax
normalization uses reciprocal_approx_fast on an SBUF copy of the sums row,
then a rank-1 broadcast matmul. The PE is pre-warmed with dummy matmuls
during the input-DMA ramp so the HAM clock gate opens early.
"""

import sys

for _p in ("/opt/trn_rl_repo",):
    if _p not in sys.path:
        sys.path.insert(0, _p)

import ml_dtypes
import numpy as np

import concourse.bacc as bacc
import concourse.mybir as mybir
import concourse.tile as tile
from concourse.alu_op_type import AluOpType
from concourse.bass_utils import run_bass_kernel_spmd

F32 = mybir.dt.float32
BF16 = mybir.dt.bfloat16
NPBF = ml_dtypes.bfloat16
EXP = mybir.ActivationFunctionType.Exp

B, T, C = 2, 2048, 1024
H, HD = 16, 64
HPC = 4          # heads per core
NPAIR = 2        # head pairs per core
CL = HPC * HD    # 256 local channels
NCORES = 8
SCALE = 0.125    # 1/sqrt(64)
MASKV = -30.0    # additive causal mask (exp(-30+smax) ~ 1e-12, negligible)

TT5 = T // 512   # 4  q supertiles
TT1 = T // 128   # 16 t tiles / k blocks
CCH = C // 128   # 8  contraction chunks
_DBG = {}


def _build_program():
    nc = bacc.Bacc("TRN2", target_bir_lowering=False, debug=False)

    xT_d = nc.dram_tensor("xT", [C, T], BF16, kind="ExternalInput").ap()
    wq_d = nc.dram_tensor("wq", [C, CL], BF16, kind="ExternalInput").ap()
    wk_d = nc.dram_tensor("wk", [C, CL], BF16, kind="ExternalInput").ap()
    wv_d = nc.dram_tensor("wv", [C, CL], BF16, kind="ExternalInput").ap()
    wp_d = nc.dram_tensor("wp", [CL, C], BF16, kind="ExternalInput").ap()
    bqs_d = nc.dram_tensor("bqs", [128, NPAIR], F32, kind="ExternalInput").ap()
    bks_d = nc.dram_tensor("bks", [128, NPAIR], F32, kind="ExternalInput").ap()
    bvr_d = nc.dram_tensor("bvr", [1, CL], BF16, kind="ExternalInput").ap()
    ones1_d = nc.dram_tensor("ones1", [1, 128], BF16, kind="ExternalInput").ap()
    idn_d = nc.dram_tensor("idn", [128, 128], BF16, kind="ExternalInput").ap()
    mneg_d = nc.dram_tensor("mneg", [128, 128], BF16, kind="ExternalInput").ap()
    yp_d = nc.dram_tensor("yp", [T, C], F32, kind="ExternalOutput").ap()

    with tile.TileContext(nc) as tc:
        _attn_kernel(tc, xT_d, wq_d, wk_d, wv_d, wp_d, bqs_d, bks_d, bvr_d,
                     ones1_d, idn_d, mneg_d, yp_d)
    nc.compile()
    return nc


def _attn_kernel(tc, xT_d, wq_d, wk_d, wv_d, wp_d, bqs_d, bks_d, bvr_d,
                 ones1_d, idn_d, mneg_d, yp_d):
    nc = tc.nc
    mm = nc.tensor.matmul

    with (
        tc.tile_pool(name="const", bufs=1) as cpool,
        tc.tile_pool(name="big", bufs=1) as bigpool,
        tc.tile_pool(name="work", bufs=2) as wkpool,
        tc.tile_pool(name="mix", bufs=1, space="PSUM") as mixps,
        tc.tile_pool(name="psy", bufs=1, space="PSUM") as psy,
        tc.tile_pool(name="pso", bufs=2, space="PSUM") as pso_pool,
    ):
        # ---- constants ----
        bqs = cpool.tile([128, NPAIR], F32)
        nc.sync.dma_start(bqs, bqs_d)
        bks = cpool.tile([128, NPAIR], F32)
        nc.sync.dma_start(bks, bks_d)
        bvr = cpool.tile([1, CL], BF16)
        nc.sync.dma_start(bvr, bvr_d)
        ones1 = cpool.tile([1, 128], BF16)
        nc.sync.dma_start(ones1, ones1_d)
        idn = cpool.tile([128, 128], BF16)
        nc.sync.dma_start(idn, idn_d)
        mneg = cpool.tile([128, 128], BF16)
        nc.sync.dma_start(mneg, mneg_d)
        ones64f = cpool.tile([1, 64], F32)
        nc.vector.memset(ones64f, 1.0)

        xt = bigpool.tile([128, CCH, T], BF16)          # x^T chunks
        wqt = bigpool.tile([128, CCH, CL], BF16)
        wkt = bigpool.tile([128, CCH, CL], BF16)
        wvt = bigpool.tile([128, CCH, CL], BF16)
        # DMA order: wq + first-half x so QKV(0) starts early; issue
        # alternates between the two HWDGE engines (sync, scalar).
        for c in range(CCH):
            nc.sync.dma_start(wqt[:, c, :], wq_d[c * 128:(c + 1) * 128, :])
            nc.sync.dma_start(xt[:, c, 0:1024],
                              xT_d[c * 128:(c + 1) * 128, 0:1024])
        for c in range(CCH):
            nc.sync.dma_start(wkt[:, c, :], wk_d[c * 128:(c + 1) * 128, :])
            nc.gpsimd.dma_start(wvt[:, c, :], wv_d[c * 128:(c + 1) * 128, :])
        for c in range(CCH):
            nc.gpsimd.dma_start(xt[:, c, 1024:2048],
                                xT_d[c * 128:(c + 1) * 128, 1024:2048])
        wpt = bigpool.tile([128, NPAIR, C], BF16)       # proj weight chunks
        for p in range(NPAIR):
            nc.sync.dma_start(wpt[:, p, :], wp_d[p * 128:(p + 1) * 128, :])

        # ---- persistent activations ----
        qt = bigpool.tile([128, NPAIR, T], BF16)        # q^T (scaled, biased)
        kt = bigpool.tile([128, NPAIR, T], BF16)        # k^T (biased)
        vt = bigpool.tile([128, TT1, HPC, HD + 1], BF16)  # v natural + ones col
        yt = bigpool.tile([128, NPAIR, T], BF16)        # attn out ^T (normalized)
        _DBG.update(qt=qt, kt=kt, vt=vt, yt=yt)

        for tt in range(TT1):
            nc.vector.memset(vt[:, tt, :, HD:HD + 1], 1.0)

        # pre-warm the PE (HAM) with dummy matmuls while the input DMAs
        # stream; writes a scratch slot that the first real ytps reuses.
        warm = psy.tile([HD + 1, 512], F32, tag="yt0", name="warm")
        for w in range(96):
            mm(warm[:, (w % 4) * 128:(w % 4) * 128 + 128],
               idn[:, 0:HD + 1], mneg, start=True, stop=True)

        # ---- QKV building blocks (emitted interleaved with attention) ----
        def emit_qk(t5, p, which):
            w_sb, dst, scale, bias = (
                (wqt, qt, SCALE, bqs) if which == "q" else (wkt, kt, 1.0, bks))
            pst = mixps.tile([128, 512], F32, tag="mx", bufs=2,
                             name=f"psqk_{t5}_{p}_{which}")
            for c in range(CCH):
                mm(pst,
                   w_sb[:, c, p * 128:(p + 1) * 128],
                   xt[:, c, t5 * 512:(t5 + 1) * 512],
                   start=(c == 0), stop=(c == CCH - 1))
            nc.vector.tensor_scalar(
                dst[:, p, t5 * 512:(t5 + 1) * 512], pst,
                scale, bias[:, p:p + 1],
                AluOpType.mult, AluOpType.add)

        def emit_v(tt):
            psv = mixps.tile([128, HPC, HD], F32, tag="mx", bufs=2,
                             name=f"psv_{tt}")
            for c in range(CCH):
                mm(psv,
                   xt[:, c, tt * 128:(tt + 1) * 128],
                   wvt[:, c, :],
                   start=(c == 0), stop=False)
            mm(psv, ones1, bvr, start=False, stop=True)
            nc.vector.tensor_copy(vt[:, tt, :, 0:HD], psv)

        def qkv_blocks(t5):
            blocks = []
            for p in range(NPAIR):
                blocks.append(lambda p=p: emit_qk(t5, p, "q"))
                blocks.append(lambda p=p: emit_qk(t5, p, "k"))
            for tt in range(4 * t5, 4 * t5 + 4):
                blocks.append(lambda tt=tt: emit_v(tt))
            return blocks

        # prologue: QKV for t5=0
        for blk in qkv_blocks(0):
            blk()

        # ---- attention with QKV(t5+1) interleaved ----
        for qst in range(TT5):
            q0 = qst * 512
            nkb = 4 * qst + 4
            fill = qkv_blocks(qst + 1) if qst + 1 < TT5 else []
            if qst == TT5 - 1:
                def proj_block(tt, nh):
                    def emit():
                        pso = pso_pool.tile([128, 512], F32, tag="pso",
                                            name=f"pso_{tt}_{nh}")
                        for p in range(NPAIR):
                            mm(pso,
                               yt[:, p, tt * 128:(tt + 1) * 128],
                               wpt[:, p, nh * 512:(nh + 1) * 512],
                               start=(p == 0), stop=(p == NPAIR - 1))
                        osb = wkpool.tile([128, 512], F32, tag="osb", bufs=3,
                                          name=f"osb_{tt}_{nh}")
                        nc.vector.tensor_copy(osb, pso)
                        nc.sync.dma_start(
                            yp_d[tt * 128:(tt + 1) * 128,
                                 nh * 512:(nh + 1) * 512], osb)
                    return emit
                fill = [proj_block(tt, nh)
                        for tt in range(4 * (qst - 2), 4 * qst)
                        for nh in range(2)]
            ngroups = NPAIR * nkb
            gidx = 0
            emitted = 0

            for p in range(NPAIR):
                ytps = [
                    psy.tile([HD + 1, 512], F32, tag=f"yt{hs}",
                             name=f"ytps_{qst}_{p}_{hs}")
                    for hs in range(2)
                ]
                exs = {}

                def emit_scores(kb, _p=p, _qst=qst, _q0=q0):
                    j = kb - 4 * _qst
                    vlo = 128 * j if j >= 0 else 0
                    diag = j >= 0
                    stf = mixps.tile([128, 2, 512], F32, tag="mx", bufs=2,
                                     name=f"stf_{_qst}_{_p}_{kb}")
                    for hs in range(2):
                        r = slice(64 * hs, 64 * hs + 64)
                        mm(stf[:, hs, vlo:512],
                           kt[r, _p, kb * 128:(kb + 1) * 128],
                           qt[r, _p, _q0 + vlo:_q0 + 512],
                           tile_position=(64 * hs, 0),
                           start=True, stop=not diag)
                    if diag:
                        # fold the causal mask into the PSUM accumulation
                        for hs in range(2):
                            mm(stf[:, hs, vlo:vlo + 128], idn, mneg,
                               start=False, stop=True)
                    ex = wkpool.tile([128, 2, 512], BF16, tag="ex", bufs=8,
                                     name=f"ex_{_qst}_{_p}_{kb}")
                    nc.scalar.activation(ex[:, :, vlo:512],
                                         stf[:, :, vlo:512], EXP)
                    exs[kb] = (ex, vlo)

                def emit_av(kb, _p=p, _qst=qst, _nkb=nkb):
                    ex, vlo = exs.pop(kb)
                    for hs in range(2):
                        mm(ytps[hs][:, vlo:512],
                           vt[:, kb, 2 * _p + hs, :],
                           ex[:, hs, vlo:512],
                           start=(kb == 0), stop=(kb == _nkb - 1))

                for kb0 in range(min(2, nkb)):
                    emit_scores(kb0)
                navail = max(len(fill) - 4, 0)
                for kb in range(nkb):
                    if kb + 2 < nkb:
                        emit_scores(kb + 2)
                    emit_av(kb)
                    gidx += 1
                    # spread the next-supertile QKV blocks over the groups
                    want = (gidx * navail) // ngroups if fill else 0
                    while emitted < want:
                        fill[emitted]()
                        emitted += 1

                # softmax normalization
                sinvs = []
                for hs in range(2):
                    srow = wkpool.tile([1, 512], F32, tag=f"srow{hs}",
                                       name=f"srow_{qst}_{p}_{hs}")
                    nc.vector.tensor_copy(srow, ytps[hs][HD:HD + 1, :])
                    sinv = wkpool.tile([1, 512], F32, tag=f"sinv{hs}",
                                       name=f"sinv_{qst}_{p}_{hs}")
                    nc.vector.reciprocal_approx_fast(sinv, srow)
                    sinvs.append(sinv)
                if p == NPAIR - 1:
                    while emitted < len(fill):
                        fill[emitted]()
                        emitted += 1
                rbs = []
                for hs in range(2):
                    rb = pso_pool.tile([64, 512], F32, tag="pso",
                                       name=f"rb_{qst}_{p}_{hs}")
                    mm(rb, ones64f, sinvs[hs], start=True, stop=True)
                    rbs.append(rb)
                for hs in range(2):
                    rbsb = wkpool.tile([64, 512], F32, tag="rbsb",
                                       name=f"rbsb_{qst}_{p}_{hs}")
                    nc.vector.tensor_copy(rbsb, rbs[hs])
                    nc.vector.tensor_mul(
                        yt[64 * hs:64 * hs + 64, p, q0:q0 + 512],
                        ytps[hs][0:HD, :], rbsb)

            # proj for the finished t-tiles (qst 1+2's are attention(3) fill)
            if qst in (TT5 - 3, TT5 - 2):
                continue
            for tt in range(4 * qst, 4 * qst + 4):
                for nh in range(2):
                    pso = pso_pool.tile([128, 512], F32, tag="pso",
                                        name=f"pso_{tt}_{nh}")
                    for p in range(NPAIR):
                        mm(pso,
                           yt[:, p, tt * 128:(tt + 1) * 128],
                           wpt[:, p, nh * 512:(nh + 1) * 512],
                           start=(p == 0), stop=(p == NPAIR - 1))
                    osb = wkpool.tile([128, 512], F32, tag="osb", bufs=3,
                                      name=f"osb_{tt}_{nh}")
                    nc.vector.tensor_copy(osb, pso)
                    nc.sync.dma_start(
                        yp_d[tt * 128:(tt + 1) * 128,
                             nh * 512:(nh + 1) * 512], osb)


_CACHE = {}


def _get_nc():
    if "nc" not in _CACHE:
        _CACHE["nc"] = _build_program()
    return _CACHE["nc"]


def make_in_maps(x, w_attn, b_attn):
    """Shard the full inputs into 8 per-core input maps."""
    x = np.asarray(x, dtype=np.float32)
    w_attn = np.asarray(w_attn, dtype=np.float32)
    b_attn = np.asarray(b_attn, dtype=np.float32)
    ones1 = np.ones((1, 128), dtype=NPBF)
    idn = np.eye(128, dtype=NPBF)
    mneg = (MASKV * (np.arange(128)[None, :] < np.arange(128)[:, None])
            ).astype(NPBF)
    in_maps = []
    for core in range(NCORES):
        b, g = divmod(core, 4)
        cs = slice(g * CL, (g + 1) * CL)
        ks = slice(C + g * CL, C + (g + 1) * CL)
        vs = slice(2 * C + g * CL, 2 * C + (g + 1) * CL)
        in_maps.append({
            "xT": np.ascontiguousarray(x[b].T).astype(NPBF),
            "wq": np.ascontiguousarray(w_attn[:, cs]).astype(NPBF),
            "wk": np.ascontiguousarray(w_attn[:, ks]).astype(NPBF),
            "wv": np.ascontiguousarray(w_attn[:, vs]).astype(NPBF),
            "wp": None,  # filled by caller (needs w_proj)
            "bqs": np.ascontiguousarray(
                (SCALE * b_attn[cs]).reshape(NPAIR, 128).T),
            "bks": np.ascontiguousarray(b_attn[ks].reshape(NPAIR, 128).T),
            "bvr": b_attn[vs].reshape(1, CL).astype(NPBF),
            "ones1": ones1,
            "idn": idn,
            "mneg": mneg,
        })
    return in_maps


def kernel(x, w_attn, b_attn, w_proj, b_proj, _trace=False):
    w_proj = np.asarray(w_proj, dtype=np.float32)
    b_proj = np.asarray(b_proj, dtype=np.float32)
    in_maps = make_in_maps(x, w_attn, b_attn)
    for core in range(NCORES):
        g = core % 4
        in_maps[core]["wp"] = np.ascontiguousarray(
            w_proj[g * CL:(g + 1) * CL, :]).astype(NPBF)
    nc = _get_nc()
    res = run_bass_kernel_spmd(nc, in_maps, core_ids=list(range(NCORES)),
                               trace=_trace)
    out = np.zeros((B, T, C), dtype=np.float32)
    for core in range(NCORES):
        out[core // 4] += res.results[core]["yp"]
    out += b_proj
    if _trace:
        kernel.last_result = res
    return out
